# revision 35
# baseline (speedup 1.0000x reference)
"""Bass/Trainium2 kernel for nn_HMEClassification (hierarchical mixture-of-experts).

Strategy: pure data parallel across 8 cores (batch sharded). Per core, per
512-wide b-tile:
  L1 (7 units = 3 gates + 4 experts), h-major: weight-stationary bf16
      matmuls lhsT=W1 block [128d,128h], rhs=xT tile [128d,512b] ->
      PSUM [128h,512b] fp32. PSUM tiles are allocated 2 banks wide so
      evacuation (relu -> SBUF bf16) runs as 16 ops/tile instead of 28,
      round-robined across ScalarE / VectorE / GpSimd to balance engines.
  L2 experts, b-major: lhsT = hs[3+e,k][:, bb*128:+128] (hidden tile slice,
      [128h,128b]), rhs = eW2 chunk [128h,64c] -> psT [128b, 64c] per
      (expert, b-block), K-accumulated over 4 h-chunks. Full 128-wide PE
      rows AND only 64 moving columns: half the PE time of the h-major
      formulation.
  Gates, b-major: rhs = [-d_u, +d_u] chunk [128h, 2] -> psG [128b, 2] per
      (gate, b-block). sigmoid(x) = 1/(1+exp(-x)) computed from the exp'd
      +-logits, so ScalarE only ever uses the exp act table (no table
      thrash with Relu/Exp/Copy; Sigmoid would force a reload).
  Softmax over classes, b-major: exp on ScalarE ([128b, 256c] per psT
      half), denominators via windowed free-dim tensor_reduce on VectorE,
      combine coefficients C_e = root*gate/S_e as per-partition scalars,
      final out[b,c] = sum_e C_e[b]*exp_e[b,c] via a scalar_tensor_tensor
      chain. No cross-partition moves, no DMA broadcast, output is
      produced directly in [b, c] row-major layout.
"""

import ml_dtypes
import numpy as np

import concourse.bass as bass
import concourse.mybir as mybir
import concourse.tile as tile
from concourse import bacc
from concourse.bass_utils import run_bass_kernel_spmd

B, D, H, C = 131072, 128, 512, 64
NCORES = 8
BC = B // NCORES        # rows per core
TB = 512                # b-tile width
KH = H // 128           # 4 h-chunks of 128
NBB = TB // 128         # 4 b-blocks per tile

F32 = mybir.dt.float32
BF16 = mybir.dt.bfloat16

# ---- bf16 consts layout (columns in [128, NB] bf16 tensor) ----
W1_OFF = 0                       # 7 units * 512 = 3584
W2_OFF = W1_OFF + 7 * H          # 16 blocks (k*4+e) * 64 = 1024
GP_OFF = W2_OFF + 16 * 64        # 3 gates * 4 chunks * 2 = 24
ONES_OFF = GP_OFF + 24           # 128 cols of ones on partition 0 (bias lhsT)
ZR_OFF = ONES_OFF + 128          # 128 cols of zeros (PSUM-zeroing lhsT)
GB_OFF = ZR_OFF + 128            # 3 * 2 cols: [-db_u, db_u] on partition 0
EB_OFF = GB_OFF + 6              # 4 * 64 cols: eb2[e] on partition 0
NB = EB_OFF + 256

# L1 evac engine assignment: 28 (u,k) slots per tile -> Act 11 / DVE 7 / Pool 10
EVAC = ["A", "D", "P", "A", "D", "P", "A",
        "P", "A", "P", "D", "A", "P", "A",
        "P", "A", "D", "P", "A", "D", "P",
        "A", "D", "P", "A", "P", "D", "A"]


def _build_consts(gW1, gW2, gb2, eW1, eW2, eb2):
    cb = np.zeros((128, NB), dtype=np.float32)
    for u in range(3):
        cb[:, W1_OFF + u * H: W1_OFF + (u + 1) * H] = gW1[u]
    for e in range(4):
        cb[:, W1_OFF + (3 + e) * H: W1_OFF + (4 + e) * H] = eW1[e]
    for k in range(KH):
        for e in range(4):
            cb[:, W2_OFF + (k * 4 + e) * 64: W2_OFF + (k * 4 + e + 1) * 64] = \
                eW2[e, k * 128:(k + 1) * 128, :]
    v = gW2[:, :, 0] - gW2[:, :, 1]          # [3, 512]
    for u in range(3):
        for k in range(KH):
            sl = slice(k * 128, (k + 1) * 128)
            a = GP_OFF + (u * KH + k) * 2
            cb[:, a] = -v[u, sl]
            cb[:, a + 1] = v[u, sl]
    cb[0, ONES_OFF: ONES_OFF + 128] = 1.0
    db = gb2[:, 0] - gb2[:, 1]               # [3]
    for u in range(3):
        cb[0, GB_OFF + 2 * u] = -db[u]
        cb[0, GB_OFF + 2 * u + 1] = db[u]
    for e in range(4):
        cb[0, EB_OFF + e * 64: EB_OFF + (e + 1) * 64] = eb2[e]
    return cb.astype(ml_dtypes.bfloat16)


def _build_b1(gb1, eb1):
    b1 = np.concatenate([gb1, eb1], axis=0)  # [7, 512]
    cf = np.zeros((128, 28), dtype=np.float32)
    for u in range(7):
        for k in range(KH):
            cf[:, u * KH + k] = b1[u, k * 128:(k + 1) * 128]
    return cf


def _ap3(t_ap, dims):
    """AP with the partition dim of t_ap and custom free dims."""
    return bass.AP(tensor=t_ap.tensor, offset=t_ap.offset,
                   ap=[list(t_ap.ap[0])] + [list(d) for d in dims])


def _build_nc(n_tiles, has_b1, has_gb2, has_eb2):
    nc = bacc.Bacc("TRN2", target_bir_lowering=False)
    xt = nc.dram_tensor("xt", [D, BC], BF16, kind="ExternalInput")
    cbd = nc.dram_tensor("cb", [128, NB], BF16, kind="ExternalInput")
    cfd = nc.dram_tensor("cf", [128, 28], F32, kind="ExternalInput")
    outD = nc.dram_tensor("outD", [BC, C], F32, kind="ExternalOutput")

    AF = mybir.ActivationFunctionType
    OP = mybir.AluOpType
    AX = mybir.AxisListType

    with tile.TileContext(nc) as tc:
        with (
            tc.tile_pool(name="singles", bufs=1) as singles,
            tc.tile_pool(name="xp", bufs=3) as xp,
            tc.tile_pool(name="hp", bufs=3) as hp,
            tc.tile_pool(name="ep", bufs=3) as ep,
            tc.tile_pool(name="sp", bufs=3) as sp,
            tc.tile_pool(name="op", bufs=2) as op_pool,
            tc.tile_pool(name="psA", bufs=2, space="PSUM") as psAp,
            tc.tile_pool(name="psD", bufs=2, space="PSUM") as psDp,
            tc.tile_pool(name="psT", bufs=1, space="PSUM") as psTp,
            tc.tile_pool(name="psG", bufs=1, space="PSUM") as psGp,
        ):
            cs = singles.tile([128, NB], BF16)
            nc.sync.dma_start(out=cs, in_=cbd[:, :])
            cf = singles.tile([128, 28], F32)
            nc.sync.dma_start(out=cf, in_=cfd[:, :])

            def w1_ap(u, k):
                a = W1_OFF + u * H + k * 128
                return cs[:, a: a + 128]

            def w2_ap(k, e):
                a = W2_OFF + (k * 4 + e) * 64
                return cs[:, a: a + 64]

            def gp_ap(u, k):
                a = GP_OFF + (u * KH + k) * 2
                return cs[:, a: a + 2]

            def evac(engine, h, ps, u, k):
                # PSUM readers must be ScalarE or VectorE (GPSIMD and DMA
                # have no PSUM route on TRN2).
                if has_b1:
                    bias = cf[:, u * KH + k: u * KH + k + 1]
                    if engine == "A":
                        nc.scalar.activation(h, ps, AF.Relu, bias=bias)
                    else:
                        nc.vector.tensor_scalar(h, ps, bias, 0.0,
                                                op0=OP.add, op1=OP.max)
                else:
                    if engine == "A":
                        nc.scalar.activation(h, ps, AF.Relu)
                    else:
                        nc.vector.tensor_scalar_max(h, ps, 0.0)

            def fetch_x(t):
                xtile = xp.tile([D, TB], BF16, tag="x", name=f"xt{t}")
                nc.sync.dma_start(out=xtile, in_=xt[:, t * TB:(t + 1) * TB])
                return xtile

            xtile = fetch_x(0)
            for t in range(n_tiles):
                # prefetch next x before this tile's output DMAs hit the
                # in-order SP queue
                xnext = fetch_x(t + 1) if t + 1 < n_tiles else None

                # ---- L1: 7 units x 4 h-chunks, h-major ----
                # Per k: units (0,1) and (2,3) fill 2-bank psA tiles (one
                # wide ScalarE evac each); units 4,5,6 fill psD banks (DVE).
                hs = {}   # (u, k) -> SBUF AP [128h, TB] bf16
                for k in range(KH):
                    for p in range(2):
                        u0, u1 = 2 * p, 2 * p + 1
                        ps = psAp.tile([128, 2 * TB], F32, tag="l1a")
                        nc.tensor.matmul(ps[:, 0:TB], w1_ap(u0, k), xtile,
                                         start=True, stop=True)
                        nc.tensor.matmul(ps[:, TB:2 * TB], w1_ap(u1, k),
                                         xtile, start=True, stop=True)
                        hw = hp.tile([128, 2 * TB], BF16, tag=f"hw{p}_{k}")
                        if has_b1:
                            evac("A", hw[:, 0:TB], ps[:, 0:TB], u0, k)
                            evac("A", hw[:, TB:2 * TB], ps[:, TB:2 * TB],
                                 u1, k)
                        else:
                            nc.scalar.activation(hw, ps, AF.Relu)
                        hs[u0, k] = hw[:, 0:TB]
                        hs[u1, k] = hw[:, TB:2 * TB]
                    for u in (4, 5, 6):
                        ps = psDp.tile([128, TB], F32, tag="l1d")
                        nc.tensor.matmul(ps, w1_ap(u, k), xtile,
                                         start=True, stop=True)
                        h = hp.tile([128, TB], BF16, tag=f"h{u}_{k}")
                        evac("D", h, ps, u, k)
                        hs[u, k] = h[:, :]

                # ---- gates + L2 experts, b-major, two waves over one psT
                # bank; per-wave exp / denominators / combine / store ----
                # PSUM "zero regions" are 2KB per partition (a whole bank
                # row), so only one start/stop group may live per bank.
                # Instead, zero each bank with a K=1 zero-weight matmul and
                # make every real matmul a pure accumulate (start=False,
                # skip_group_check) — same-engine WAW deps keep order.
                zrow = cs[0:1, ZR_OFF:ZR_OFF + 128]
                ones = cs[0:1, ONES_OFF:ONES_OFF + 128]
                psG = psGp.tile([128, 32], F32, tag="g")
                nc.tensor.matmul(psG, zrow, cs[0:1, 0:32],
                                 start=True, stop=True)
                for h2 in range(2):
                    psT = psTp.tile([128, TB], F32, tag="t")
                    nc.tensor.matmul(psT, zrow, cs[0:1, 0:TB],
                                     start=True, stop=True)
                    for k in range(KH):
                        for sub in range(2):
                            bb = h2 * 2 + sub
                            bsl = slice(bb * 128, (bb + 1) * 128)
                            for u in range(3):
                                go = bb * 8 + u * 2
                                nc.tensor.matmul(psG[:, go:go + 2],
                                                 hs[u, k][:, bsl],
                                                 gp_ap(u, k),
                                                 start=False, stop=False,
                                                 skip_group_check=True)
                            half = sub * 256
                            for e in range(4):
                                eo = half + e * 64
                                nc.tensor.matmul(psT[:, eo:eo + 64],
                                                 hs[3 + e, k][:, bsl],
                                                 w2_ap(k, e),
                                                 start=False, stop=False,
                                                 skip_group_check=True)
                    for sub in range(2):
                        bb = h2 * 2 + sub
                        if has_gb2:
                            for u in range(3):
                                go = bb * 8 + u * 2
                                nc.tensor.matmul(
                                    psG[:, go:go + 2], ones,
                                    cs[0:1, GB_OFF + 2 * u:GB_OFF + 2 * u + 2],
                                    start=False, stop=False,
                                    skip_group_check=True)
                        if has_eb2:
                            half = sub * 256
                            for e in range(4):
                                eo = half + e * 64
                                nc.tensor.matmul(
                                    psT[:, eo:eo + 64], ones,
                                    cs[0:1, EB_OFF + e * 64:EB_OFF + (e + 1) * 64],
                                    start=False, stop=False,
                                    skip_group_check=True)

                    # exp of expert logits for this wave
                    expT = ep.tile([128, TB], BF16, tag=f"e{h2}")
                    nc.scalar.activation(expT, psT, AF.Exp)

                    # gate sigmoids for this wave's two b-blocks:
                    # sigma(z) = 1/(1+e^-z); psG cols already hold -+z pairs
                    gE = sp.tile([128, 16], F32, tag=f"gE{h2}")
                    nc.scalar.activation(gE, psG[:, h2 * 16:(h2 + 1) * 16],
                                         AF.Exp)
                    gF = sp.tile([128, 16], F32, tag=f"gF{h2}")
                    nc.gpsimd.tensor_scalar_add(gF, gE, 1.0)
                    gR = sp.tile([128, 16], F32, tag=f"gR{h2}")
                    nc.vector.reciprocal(gR, gF)
                    # gR cols per bb (stride 8): [rA, rB, gA0, gA1, gB0, gB1]
                    pAll = sp.tile([128, 8], F32, tag=f"pA{h2}")
                    root_ap = _ap3(gR, [[8, 2], [1, 2], [0, 2]])
                    gate_ap = _ap3(gR[:, 2:3], [[8, 2], [2, 2], [1, 2]])
                    out_ap = _ap3(pAll, [[4, 2], [2, 2], [1, 2]])
                    nc.gpsimd.tensor_tensor(out_ap, root_ap, gate_ap,
                                            op=OP.mult)

                    # softmax denominators and combine coefficients
                    sc = sp.tile([128, 8], F32, tag=f"s{h2}")
                    nc.vector.tensor_reduce(sc, _ap3(expT, [[64, 8], [1, 64]]),
                                            axis=AX.X, op=OP.add)
                    sr = sp.tile([128, 8], F32, tag=f"sr{h2}")
                    nc.vector.reciprocal(sr, sc)
                    cc = sp.tile([128, 8], F32, tag=f"c{h2}")
                    nc.vector.tensor_tensor(cc, pAll, sr, op=OP.mult)

                    # q = expT * C (C broadcast over each expert's 64 class
                    # cols via stride-0 AP), then sum the 4 experts with a
                    # 2-level TT-add tree on GpSimd (SBUF-only engine).
                    q = op_pool.tile([128, TB], F32, tag=f"q{h2}")
                    c_bc = _ap3(cc, [[1, 8], [0, 64]])
                    q_ap = _ap3(q, [[64, 8], [1, 64]])
                    pe_ap = _ap3(expT, [[64, 8], [1, 64]])
                    nc.gpsimd.tensor_tensor(q_ap, pe_ap, c_bc, op=OP.mult)
                    s1 = op_pool.tile([128, 256], F32, tag=f"s1{h2}")
                    nc.gpsimd.tensor_tensor(
                        s1, _ap3(q, [[256, 2], [1, 128]]),
                        _ap3(q[:, 128:129], [[256, 2], [1, 128]]),
                        op=OP.add)
                    o = op_pool.tile([128, 128], F32, tag=f"o{h2}")
                    nc.gpsimd.tensor_tensor(
                        o, _ap3(s1, [[128, 2], [1, 64]]),
                        _ap3(s1[:, 64:65], [[128, 2], [1, 64]]),
                        op=OP.add)
                    ob = outD[t * TB + h2 * 256: t * TB + h2 * 256 + 1, :]
                    dst = bass.AP(tensor=ob.tensor, offset=ob.offset,
                                  ap=[[C, 128], [128 * C, 2], [1, C]])
                    nc.sync.dma_start(out=dst, in_=_ap3(o, [[64, 2], [1, 64]]))

                xtile = xnext

    nc.compile()
    return nc


def kernel(x, gW1, gb1, gW2, gb2, eW1, eb1, eW2, eb2, _trace=False):
    x = np.asarray(x, dtype=np.float32)
    gW1 = np.asarray(gW1, np.float32)
    gb1 = np.asarray(gb1, np.float32)
    gW2 = np.asarray(gW2, np.float32)
    gb2 = np.asarray(gb2, np.float32)
    eW1 = np.asarray(eW1, np.float32)
    eb1 = np.asarray(eb1, np.float32)
    eW2 = np.asarray(eW2, np.float32)
    eb2 = np.asarray(eb2, np.float32)

    cb = _build_consts(gW1, gW2, gb2, eW1, eW2, eb2)
    cf = _build_b1(gb1, eb1)
    has_b1 = bool(np.any(gb1) or np.any(eb1))
    has_gb2 = bool(np.any(gb2[:, 0] - gb2[:, 1]))
    has_eb2 = bool(np.any(eb2))

    n_rows = x.shape[0]
    bc = n_rows // NCORES
    n_tiles = bc // TB
    assert bc * NCORES == n_rows and n_tiles * TB == bc

    global BC
    BC = bc
    nc = _build_nc(n_tiles, has_b1, has_gb2, has_eb2)

    xs = x.reshape(NCORES, bc, D)
    in_maps = [
        {"xt": np.ascontiguousarray(xs[c].T).astype(ml_dtypes.bfloat16),
         "cb": cb, "cf": cf}
        for c in range(NCORES)
    ]
    res = run_bass_kernel_spmd(nc, in_maps, core_ids=list(range(NCORES)),
                               trace=_trace)
    out = np.concatenate([r["outD"] for r in res.results], axis=0)
    kernel.last_results = res
    return np.ascontiguousarray(out.astype(np.float32))
